# revision 20
# baseline (speedup 1.0000x reference)
"""ARMA multi-head attention TRN2 kernel (nn_ARMAMultiHeadAttention).

Problem: B=2, S=2048, D=1024, H=16 heads of depth 64, causal mask.
  qh/kh/vh = split_heads(x @ W + b);  logits = qh@kh^T/8 + mask*(-1e9)
  (+ alpha*ar + beta*ma per-QUERY-row bias, which is softmax-shift-invariant
   and therefore has no effect on the outputs -> skipped on device)
  attn = softmax(logits);  out = (attn @ vh merged) @ wo + wo_b
Returns (out, attn) like the reference.

Sharding: 8 cores; core c handles batch b=c//4 and heads 4*(c%4)..4*(c%4)+3
(data + head parallel). wq/wk/wv column-sharded, wo row-sharded; per-batch
partial outputs summed on the host (the unshard step of row-parallel wo).

Numerics: fp16 on the PE-heavy paths (1 cyc/row vs 4 for fp32; values all
within fp16 range; ~2.4e-4 rounding), fp32 PSUM accumulation everywhere,
fp32 softmax reciprocal. Mask bias is -6e4 (fits fp16; exp underflows to 0
exactly, same as the reference's -1e9). attn is stored fp16 and widened on
the host.

Per-core flow:
  - projections: qh^T,kh^T [256,2048] fp16; vh [2048, 4 heads x 66] fp16
    with a ones column per head (fuses the softmax row-sum into attn@vh)
  - per (q-tile, head pair): causal logits chunks in PSUM (two heads
    interleaved on disjoint PE row groups), -6e4 on the diagonal block via
    an identity matmul, Exp -> e_t fp16, PE-transpose of e blocks -> attn@vh
    (N=66: col 64 = row sum), reciprocal, normalize+store attn, scale
    context, context^T, wo row-shard matmul streamed per q-tile.
"""

import os
import sys
import types
import numpy as np
import ml_dtypes

import concourse.bass as bass
import concourse.mybir as mybir
import concourse.tile as tile
from concourse import bacc
from concourse.bass_utils import run_bass_kernel_spmd
from concourse.masks import make_identity

F32 = mybir.dt.float32
F16 = mybir.dt.float16
AF = mybir.ActivationFunctionType

B, S, D, H = 2, 2048, 1024, 16
DEPTH = D // H          # 64
NCORES = 8
HPC = H // 4            # heads per core = 4
DH = HPC * DEPTH        # 256 per-core projection width
NQT = S // 128          # 16 q tiles
KC = D // 128           # 8 contraction chunks for projections
CHUNK = 1024            # logits psum chunk (2 banks)
VW = 66                 # vh cols per head: 64 data + ones + pad
MASKVAL = -60000.0      # "-inf" that fits fp16; exp(0.125*x) underflows to 0

_CACHE = {}


def _install_ntff_hook():
    """The agent image's antenv lacks axon_hooks; register the NTFF profile
    hook manually so run_bass_kernel_spmd(trace=True) works under axon."""
    import antenv
    if "antenv.axon_hooks" in sys.modules:
        return
    mod = types.ModuleType("antenv.axon_hooks")
    _hook = [None]
    mod.set_axon_ntff_profile_hook = lambda h: _hook.__setitem__(0, h)
    mod.get_axon_ntff_profile_hook = lambda: _hook[0]
    sys.modules["antenv.axon_hooks"] = mod
    antenv.axon_hooks = mod
    from trn_agent_boot.trn_boot import _ntff_profile_via_ctypes
    mod.set_axon_ntff_profile_hook(
        _ntff_profile_via_ctypes("/opt/axon/libaxon_pjrt.so"))


def _build():
    nc = bacc.Bacc("TRN2", target_bir_lowering=False, debug=False,
                   enable_asserts=False)

    xqT = nc.dram_tensor("xqT", [D, S], F16, kind="ExternalInput").ap()
    xkT = nc.dram_tensor("xkT", [D, S], F16, kind="ExternalInput").ap()
    xvT = nc.dram_tensor("xvT", [D, S], F16, kind="ExternalInput").ap()
    wq = nc.dram_tensor("wq", [D, DH], F16, kind="ExternalInput").ap()
    wk = nc.dram_tensor("wk", [D, DH], F16, kind="ExternalInput").ap()
    wv = nc.dram_tensor("wv", [D, DH], F16, kind="ExternalInput").ap()
    wqb = nc.dram_tensor("wqb", [2, 128], F32, kind="ExternalInput").ap()
    wkb = nc.dram_tensor("wkb", [2, 128], F32, kind="ExternalInput").ap()
    wvb = nc.dram_tensor("wvb", [1, DH], F16, kind="ExternalInput").ap()
    wo = nc.dram_tensor("wo", [2, 128, D], F16, kind="ExternalInput").ap()
    maskd = nc.dram_tensor("maskd", [128, 128], F16, kind="ExternalInput").ap()

    attn = nc.dram_tensor("attn", [HPC, S, S], F16, kind="ExternalOutput").ap()
    pout = nc.dram_tensor("pout", [S, D], F32, kind="ExternalOutput").ap()

    with tile.TileContext(nc) as tc:
        with (
            tc.tile_pool(name="persist", bufs=1) as persist,
            tc.tile_pool(name="small", bufs=1) as small,
        ):
            qhT = persist.tile([128, 2, S], F16, tag="qhT")
            khT = persist.tile([128, 2, S], F16, tag="khT")
            vh = persist.tile([128, NQT, HPC, VW], F16, tag="vh")
            ctT = persist.tile([128, 2, S], F16, tag="ctT")
            wo_sb = persist.tile([128, 2, D], F16, tag="wo")

            ident = small.tile([128, 128], F32)
            make_identity(nc, ident[:])
            ident_h = small.tile([128, 128], F16)
            make_identity(nc, ident_h[:])
            maskd_sb = small.tile([128, 128], F16)
            nc.sync.dma_start(maskd_sb[:], maskd)
            wqb_sb = small.tile([128, 2], F32)
            nc.sync.dma_start(wqb_sb[:], wqb.rearrange("j p -> p j"))
            wkb_sb = small.tile([128, 2], F32)
            nc.sync.dma_start(wkb_sb[:], wkb.rearrange("j p -> p j"))
            wvb_sb = small.tile([1, DH], F16)
            nc.sync.dma_start(wvb_sb[:], wvb)
            ones1 = small.tile([1, 128], F16)
            nc.vector.memset(ones1[:], 1.0)
            nc.sync.dma_start(wo_sb[:], wo.rearrange("j p n -> p j n"))
            # ones column (and pad) for the fused row-sum
            nc.vector.memset(vh[:, :, :, 64:65], 1.0)
            nc.vector.memset(vh[:, :, :, 65:66], 0.0)

            # ---------------- projections ----------------
            with (
                tc.tile_pool(name="xin", bufs=1) as xin,
                tc.tile_pool(name="wproj", bufs=1) as wproj,
                tc.tile_pool(name="ppsum", bufs=8, space="PSUM") as ppsum,
            ):
                wq_sb = wproj.tile([128, KC, DH], F16, tag="wq")
                wk_sb = wproj.tile([128, KC, DH], F16, tag="wk")
                wv_sb = wproj.tile([128, KC, DH], F16, tag="wv")
                nc.sync.dma_start(wq_sb[:], wq.rearrange("(c p) m -> p c m", p=128))
                nc.sync.dma_start(wk_sb[:], wk.rearrange("(c p) m -> p c m", p=128))
                nc.sync.dma_start(wv_sb[:], wv.rearrange("(c p) m -> p c m", p=128))

                for src, wsb, bsb, dstT in (
                    (xqT, wq_sb, wqb_sb, qhT),
                    (xkT, wk_sb, wkb_sb, khT),
                ):
                    x_sb = xin.tile([128, KC, S], F16, tag="x")
                    src_r = src.rearrange("(c p) s -> p c s", p=128)
                    for c in range(KC):  # chunked so matmuls start early
                        nc.sync.dma_start(x_sb[:, c, :], src_r[:, c, :])
                    # all 8 output groups accumulate in parallel (8 PSUM
                    # banks) so PE streams chunk-by-chunk as DMA lands
                    pss = [ppsum.tile([128, 512], F32, tag="p", name=f"pp{i}")
                           for i in range(8)]
                    for c in range(KC):
                        for j in range(2):
                            for n in range(S // 512):
                                nc.tensor.matmul(
                                    pss[j * 4 + n][:],
                                    wsb[:, c, j * 128:(j + 1) * 128],
                                    x_sb[:, c, n * 512:(n + 1) * 512],
                                    start=(c == 0), stop=(c == KC - 1))
                    for j in range(2):
                        for n in range(S // 512):
                            nc.scalar.activation(
                                dstT[:, j, n * 512:(n + 1) * 512],
                                pss[j * 4 + n][:],
                                AF.Identity, bias=bsb[:, j:j + 1])

                # v projection: natural layout [s, per-head 66-col groups]
                x_sb = xin.tile([128, KC, S], F16, tag="x")
                xv_r = xvT.rearrange("(c p) s -> p c s", p=128)
                for c in range(KC):
                    nc.sync.dma_start(x_sb[:, c, :], xv_r[:, c, :])
                for st in range(NQT):
                    ps = ppsum.tile([128, 512], F32, tag="p")
                    for c in range(KC):
                        nc.tensor.matmul(
                            ps[:, :DH],
                            x_sb[:, c, st * 128:(st + 1) * 128],
                            wv_sb[:, c, :],
                            start=(c == 0), stop=False)
                    nc.tensor.matmul(ps[:, :DH], ones1[:], wvb_sb[:],
                                     start=False, stop=True)
                    nc.scalar.copy(
                        vh[:, st, :, 0:64],
                        ps[:, :DH].rearrange("p (h d) -> p h d", d=64))

            # ---------------- attention ----------------
            with (
                tc.tile_pool(name="erow", bufs=3) as erow,
                tc.tile_pool(name="arow", bufs=3) as arow,
                tc.tile_pool(name="eT", bufs=4) as eTp,
                tc.tile_pool(name="stats", bufs=4) as stats,
                tc.tile_pool(name="ctile", bufs=2) as ctile,
                tc.tile_pool(name="psL", bufs=2, space="PSUM") as psL,
                tc.tile_pool(name="psT", bufs=2, space="PSUM") as psT,
                tc.tile_pool(name="psAV", bufs=2, space="PSUM") as psAV,
            ):
                copy_flip = 0
                for qt in range(NQT):
                    kext = (qt + 1) * 128
                    nblk = qt + 1
                    nch = (kext + CHUNK - 1) // CHUNK
                    c_t = ctile.tile([128, DH], F16, tag="c")
                    for g in range(2):  # head pairs (2g, 2g+1), j = g
                        e_ts = []
                        ps_ls = []
                        for hh in range(2):
                            e_ts.append(erow.tile([128, S], F16, tag="e",
                                                  name=f"e{hh}"))
                        # ---- logits chunks + exp, heads interleaved so the
                        # two K=64 matmuls run on disjoint PE row groups ----
                        for ci in range(nch):
                            k0 = ci * CHUNK
                            kw = min(CHUNK, kext - k0)
                            ps_ls = [psL.tile([128, CHUNK], F32, tag="l",
                                              name=f"l{hh}") for hh in range(2)]
                            for sub in range((kw + 511) // 512):
                                s0 = sub * 512
                                sw = min(512, kw - s0)
                                last = (ci == nch - 1) and (s0 + sw == kw)
                                for hh in range(2):
                                    po = 64 * hh
                                    nc.tensor.matmul(
                                        ps_ls[hh][:, s0:s0 + sw],
                                        qhT[po:po + 64, g, qt * 128:(qt + 1) * 128],
                                        khT[po:po + 64, g, k0 + s0:k0 + s0 + sw],
                                        start=True, stop=not last)
                                if last:
                                    for hh in range(2):
                                        nc.tensor.matmul(
                                            ps_ls[hh][:, kw - 128:kw], ident_h[:],
                                            maskd_sb[:], start=False, stop=True)
                            for hh in range(2):
                                nc.scalar.activation(
                                    e_ts[hh][:, k0:k0 + kw], ps_ls[hh][:, :kw],
                                    AF.Exp, scale=0.125)
                        # ---- transpose + attn@vh (unnormalised exp) ----
                        for hh in range(2):
                            h = 2 * g + hh
                            e_t = e_ts[hh]
                            ps_av = psAV.tile([128, VW], F32, tag="av",
                                              name=f"av{hh}")
                            for g0 in range(0, nblk, 4):
                                gn = min(4, nblk - g0)
                                ps_t = psT.tile([128, 512], F16, tag="t")
                                for bi in range(gn):
                                    nc.tensor.transpose(
                                        ps_t[:, bi * 128:(bi + 1) * 128],
                                        e_t[:, (g0 + bi) * 128:(g0 + bi + 1) * 128],
                                        ident_h[:])
                                eT_sb = eTp.tile([128, 512], F16, tag="eT")
                                nc.vector.tensor_copy(eT_sb[:, :gn * 128],
                                                      ps_t[:, :gn * 128])
                                for bi in range(gn):
                                    kb = g0 + bi
                                    nc.tensor.matmul(
                                        ps_av[:],
                                        eT_sb[:, bi * 128:(bi + 1) * 128],
                                        vh[:, kb, h, :],
                                        start=(kb == 0), stop=(kb == nblk - 1))
                            # ---- softmax tail: row sum is ps_av[:, 64] ----
                            rc = stats.tile([128, 1], F32, tag="rc")
                            nc.vector.reciprocal(rc[:], ps_av[:, 64:65])
                            a_t = arow.tile([128, S], F16, tag="a")
                            nc.gpsimd.tensor_scalar_mul(a_t[:, :kext],
                                                        e_t[:, :kext], rc[:])
                            nc.sync.dma_start(
                                attn[h, qt * 128:(qt + 1) * 128, 0:kext],
                                a_t[:, :kext])
                            nc.vector.tensor_scalar_mul(
                                c_t[:, h * 64:(h + 1) * 64], ps_av[:, 0:64], rc[:])
                    # ---- context transpose + output projection ----
                    for jj in range(2):
                        ps_ct = psT.tile([128, 512], F16, tag="t")
                        nc.tensor.transpose(
                            ps_ct[:, :128],
                            c_t[:, jj * 128:(jj + 1) * 128],
                            ident_h[:])
                        nc.scalar.copy(
                            ctT[:, jj, qt * 128:(qt + 1) * 128], ps_ct[:, :128])
                    for n in range(D // 512):
                        ps_o = psT.tile([128, 512], F32, tag="t")
                        for jj in range(2):
                            nc.tensor.matmul(
                                ps_o[:],
                                ctT[:, jj, qt * 128:(qt + 1) * 128],
                                wo_sb[:, jj, n * 512:(n + 1) * 512],
                                start=(jj == 0), stop=(jj == 1))
                        o_t = eTp.tile([128, 512], F32, tag="ot")
                        nc.scalar.copy(o_t[:], ps_o[:])
                        nc.sync.dma_start(
                            pout[qt * 128:(qt + 1) * 128, n * 512:(n + 1) * 512],
                            o_t[:])

    nc.compile()
    return nc


def _make_in_maps(v, k, q, mask, wq_k, wq_b, wk_k, wk_b, wv_k, wv_b, wo_k):
    maskd = np.ascontiguousarray(mask[0, 0, :128, :128]
                                 * np.float32(MASKVAL)).astype(np.float16)
    in_maps = []
    for c in range(NCORES):
        b = c // 4
        h0 = (c % 4) * HPC
        cs, ce = h0 * DEPTH, (h0 + HPC) * DEPTH
        in_maps.append({
            "xqT": np.ascontiguousarray(q[b].T).astype(np.float16),
            "xkT": np.ascontiguousarray(k[b].T).astype(np.float16),
            "xvT": np.ascontiguousarray(v[b].T).astype(np.float16),
            "wq": np.ascontiguousarray(wq_k[:, cs:ce]).astype(np.float16),
            "wk": np.ascontiguousarray(wk_k[:, cs:ce]).astype(np.float16),
            "wv": np.ascontiguousarray(wv_k[:, cs:ce]).astype(np.float16),
            "wqb": np.ascontiguousarray(wq_b[cs:ce].reshape(2, 128)),
            "wkb": np.ascontiguousarray(wk_b[cs:ce].reshape(2, 128)),
            "wvb": np.ascontiguousarray(wv_b[cs:ce].reshape(1, DH)).astype(np.float16),
            "wo": np.ascontiguousarray(wo_k[cs:ce].reshape(2, 128, D)).astype(np.float16),
            "maskd": maskd,
        })
    return in_maps


def kernel(v, k, q, mask, wq_k, wq_b, wk_k, wk_b, wv_k, wv_b, wo_k, wo_b,
           ar_w, ma_w, alpha, beta, _trace=False):
    v = np.asarray(v, np.float32)
    k = np.asarray(k, np.float32)
    q = np.asarray(q, np.float32)
    mask = np.asarray(mask, np.float32)
    wq_k = np.asarray(wq_k, np.float32)
    wk_k = np.asarray(wk_k, np.float32)
    wv_k = np.asarray(wv_k, np.float32)
    wo_k = np.asarray(wo_k, np.float32)
    wq_b = np.asarray(wq_b, np.float32)
    wk_b = np.asarray(wk_b, np.float32)
    wv_b = np.asarray(wv_b, np.float32)
    wo_b = np.asarray(wo_b, np.float32)

    if _trace:
        _install_ntff_hook()
    if "nc" not in _CACHE:
        _CACHE["nc"] = _build()
    nc = _CACHE["nc"]

    in_maps = _make_in_maps(v, k, q, mask, wq_k, wq_b, wk_k, wk_b,
                            wv_k, wv_b, wo_k)
    res = run_bass_kernel_spmd(nc, in_maps, list(range(NCORES)), trace=_trace)
    if _trace:
        print(f"HW exec time: {res.exec_time_ns} ns", flush=True)
        _CACHE["exec_time_ns"] = res.exec_time_ns
        _CACHE["results"] = res

    attn_full = np.concatenate(
        [res.results[c]["attn"][None].astype(np.float32) for c in range(NCORES)]
    ).reshape(B, H, S, S)
    out = np.stack([
        sum(res.results[c]["pout"] for c in range(4 * b, 4 * b + 4)) + wo_b
        for b in range(B)
    ]).astype(np.float32)
    return out, attn_full


# revision 21
# speedup vs baseline: 4.0553x; 4.0553x over previous
"""ARMA multi-head attention TRN2 kernel (nn_ARMAMultiHeadAttention).

Problem: B=2, S=2048, D=1024, H=16 heads of depth 64, causal mask.
  qh/kh/vh = split_heads(x @ W + b);  logits = qh@kh^T/8 + mask*(-1e9)
  (+ alpha*ar + beta*ma per-QUERY-row bias, which is softmax-shift-invariant
   and therefore has no effect on the outputs -> skipped on device)
  attn = softmax(logits);  out = (attn @ vh merged) @ wo + wo_b
Returns (out, attn) like the reference.

Sharding: 8 cores; core c handles batch b=c//4 and heads 4*(c%4)..4*(c%4)+3
(data + head parallel). wq/wk/wv column-sharded, wo row-sharded; per-batch
partial outputs summed on the host (the unshard step of row-parallel wo).

Numerics: fp16 on the PE-heavy paths (1 cyc/row vs 4 for fp32; values all
within fp16 range; ~2.4e-4 rounding), fp32 PSUM accumulation everywhere,
fp32 softmax reciprocal. Mask bias is -6e4 (fits fp16; exp underflows to 0
exactly, same as the reference's -1e9). attn is stored fp16 and widened on
the host.

Per-core flow:
  - projections: qh^T,kh^T [256,2048] fp16; vh [2048, 4 heads x 66] fp16
    with a ones column per head (fuses the softmax row-sum into attn@vh)
  - per (q-tile, head pair): causal logits chunks in PSUM (two heads
    interleaved on disjoint PE row groups), -6e4 on the diagonal block via
    an identity matmul, Exp -> e_t fp16, PE-transpose of e blocks -> attn@vh
    (N=66: col 64 = row sum), reciprocal, normalize+store attn, scale
    context, context^T, wo row-shard matmul streamed per q-tile.
"""

import os
import sys
import types
import numpy as np
import ml_dtypes

import concourse.bass as bass
import concourse.mybir as mybir
import concourse.tile as tile
from concourse import bacc
from concourse.bass_utils import run_bass_kernel_spmd
from concourse.masks import make_identity

F32 = mybir.dt.float32
F16 = mybir.dt.float16
AF = mybir.ActivationFunctionType

B, S, D, H = 2, 2048, 1024, 16
DEPTH = D // H          # 64
NCORES = 8
HPC = H // 4            # heads per core = 4
DH = HPC * DEPTH        # 256 per-core projection width
NQT = S // 128          # 16 q tiles
KC = D // 128           # 8 contraction chunks for projections
CHUNK = 1024            # logits psum chunk (2 banks)
VW = 66                 # vh cols per head: 64 data + ones + pad
MASKVAL = -60000.0      # "-inf" that fits fp16; exp(0.125*x) underflows to 0

_CACHE = {}


def _install_ntff_hook():
    """The agent image's antenv lacks axon_hooks; register the NTFF profile
    hook manually so run_bass_kernel_spmd(trace=True) works under axon."""
    import antenv
    if "antenv.axon_hooks" in sys.modules:
        return
    mod = types.ModuleType("antenv.axon_hooks")
    _hook = [None]
    mod.set_axon_ntff_profile_hook = lambda h: _hook.__setitem__(0, h)
    mod.get_axon_ntff_profile_hook = lambda: _hook[0]
    sys.modules["antenv.axon_hooks"] = mod
    antenv.axon_hooks = mod
    from trn_agent_boot.trn_boot import _ntff_profile_via_ctypes
    mod.set_axon_ntff_profile_hook(
        _ntff_profile_via_ctypes("/opt/axon/libaxon_pjrt.so"))


def _build():
    nc = bacc.Bacc("TRN2", target_bir_lowering=False, debug=False,
                   enable_asserts=False)

    xqT = nc.dram_tensor("xqT", [D, S], F16, kind="ExternalInput").ap()
    xkT = nc.dram_tensor("xkT", [D, S], F16, kind="ExternalInput").ap()
    xvT = nc.dram_tensor("xvT", [D, S], F16, kind="ExternalInput").ap()
    wq = nc.dram_tensor("wq", [D, DH], F16, kind="ExternalInput").ap()
    wk = nc.dram_tensor("wk", [D, DH], F16, kind="ExternalInput").ap()
    wv = nc.dram_tensor("wv", [D, DH], F16, kind="ExternalInput").ap()
    wqb = nc.dram_tensor("wqb", [2, 128], F32, kind="ExternalInput").ap()
    wkb = nc.dram_tensor("wkb", [2, 128], F32, kind="ExternalInput").ap()
    wvb = nc.dram_tensor("wvb", [1, DH], F16, kind="ExternalInput").ap()
    wo = nc.dram_tensor("wo", [2, 128, D], F16, kind="ExternalInput").ap()
    maskd = nc.dram_tensor("maskd", [128, 128], F16, kind="ExternalInput").ap()

    attn = nc.dram_tensor("attn", [HPC, S, S], F16, kind="ExternalOutput").ap()
    pout = nc.dram_tensor("pout", [S, D], F32, kind="ExternalOutput").ap()

    with tile.TileContext(nc) as tc:
        with (
            tc.tile_pool(name="persist", bufs=1) as persist,
            tc.tile_pool(name="small", bufs=1) as small,
        ):
            qhT = persist.tile([128, 2, S], F16, tag="qhT")
            khT = persist.tile([128, 2, S], F16, tag="khT")
            vh = persist.tile([128, NQT, HPC, VW], F16, tag="vh")
            ctT = persist.tile([128, 2, S], F16, tag="ctT")
            wo_sb = persist.tile([128, 2, D], F16, tag="wo")

            ident = small.tile([128, 128], F32)
            make_identity(nc, ident[:])
            ident_h = small.tile([128, 128], F16)
            make_identity(nc, ident_h[:])
            maskd_sb = small.tile([128, 128], F16)
            nc.sync.dma_start(maskd_sb[:], maskd)
            wqb_sb = small.tile([128, 2], F32)
            nc.sync.dma_start(wqb_sb[:], wqb.rearrange("j p -> p j"))
            wkb_sb = small.tile([128, 2], F32)
            nc.sync.dma_start(wkb_sb[:], wkb.rearrange("j p -> p j"))
            wvb_sb = small.tile([1, DH], F16)
            nc.sync.dma_start(wvb_sb[:], wvb)
            ones1 = small.tile([1, 128], F16)
            nc.vector.memset(ones1[:], 1.0)
            nc.sync.dma_start(wo_sb[:], wo.rearrange("j p n -> p j n"))
            # ones column (and pad) for the fused row-sum
            nc.vector.memset(vh[:, :, :, 64:65], 1.0)
            nc.vector.memset(vh[:, :, :, 65:66], 0.0)

            # ---------------- projections ----------------
            with (
                tc.tile_pool(name="xin", bufs=1) as xin,
                tc.tile_pool(name="wproj", bufs=1) as wproj,
                tc.tile_pool(name="ppsum", bufs=8, space="PSUM") as ppsum,
            ):
                wq_sb = wproj.tile([128, KC, DH], F16, tag="wq")
                wk_sb = wproj.tile([128, KC, DH], F16, tag="wk")
                wv_sb = wproj.tile([128, KC, DH], F16, tag="wv")
                nc.sync.dma_start(wq_sb[:], wq.rearrange("(c p) m -> p c m", p=128))
                nc.sync.dma_start(wk_sb[:], wk.rearrange("(c p) m -> p c m", p=128))
                nc.sync.dma_start(wv_sb[:], wv.rearrange("(c p) m -> p c m", p=128))

                for src, wsb, bsb, dstT in (
                    (xqT, wq_sb, wqb_sb, qhT),
                    (xkT, wk_sb, wkb_sb, khT),
                ):
                    x_sb = xin.tile([128, KC, S], F16, tag="x")
                    src_r = src.rearrange("(c p) s -> p c s", p=128)
                    for c in range(KC):  # chunked so matmuls start early
                        nc.sync.dma_start(x_sb[:, c, :], src_r[:, c, :])
                    # all 8 output groups accumulate in parallel (8 PSUM
                    # banks) so PE streams chunk-by-chunk as DMA lands
                    pss = [ppsum.tile([128, 512], F32, tag="p", name=f"pp{i}")
                           for i in range(8)]
                    for c in range(KC):
                        for j in range(2):
                            for n in range(S // 512):
                                nc.tensor.matmul(
                                    pss[j * 4 + n][:],
                                    wsb[:, c, j * 128:(j + 1) * 128],
                                    x_sb[:, c, n * 512:(n + 1) * 512],
                                    start=(c == 0), stop=(c == KC - 1))
                    for j in range(2):
                        for n in range(S // 512):
                            nc.scalar.activation(
                                dstT[:, j, n * 512:(n + 1) * 512],
                                pss[j * 4 + n][:],
                                AF.Identity, bias=bsb[:, j:j + 1])

                # v projection: natural layout [s, per-head 66-col groups]
                x_sb = xin.tile([128, KC, S], F16, tag="x")
                xv_r = xvT.rearrange("(c p) s -> p c s", p=128)
                for c in range(KC):
                    nc.sync.dma_start(x_sb[:, c, :], xv_r[:, c, :])
                for st in range(NQT):
                    ps = ppsum.tile([128, 512], F32, tag="p")
                    for c in range(KC):
                        nc.tensor.matmul(
                            ps[:, :DH],
                            x_sb[:, c, st * 128:(st + 1) * 128],
                            wv_sb[:, c, :],
                            start=(c == 0), stop=False)
                    nc.tensor.matmul(ps[:, :DH], ones1[:], wvb_sb[:],
                                     start=False, stop=True)
                    nc.scalar.copy(
                        vh[:, st, :, 0:64],
                        ps[:, :DH].rearrange("p (h d) -> p h d", d=64))

            # ---------------- attention ----------------
            with (
                tc.tile_pool(name="erow", bufs=3) as erow,
                tc.tile_pool(name="arow", bufs=3) as arow,
                tc.tile_pool(name="eT", bufs=4) as eTp,
                tc.tile_pool(name="stats", bufs=4) as stats,
                tc.tile_pool(name="ctile", bufs=2) as ctile,
                tc.tile_pool(name="psL", bufs=2, space="PSUM") as psL,
                tc.tile_pool(name="psT", bufs=2, space="PSUM") as psT,
                tc.tile_pool(name="psAV", bufs=2, space="PSUM") as psAV,
            ):
                copy_flip = 0
                for qt in range(NQT):
                    kext = (qt + 1) * 128
                    nblk = qt + 1
                    nch = (kext + CHUNK - 1) // CHUNK
                    c_t = ctile.tile([128, DH], F16, tag="c")
                    for g in range(2):  # head pairs (2g, 2g+1), j = g
                        e_ts = []
                        ps_ls = []
                        for hh in range(2):
                            e_ts.append(erow.tile([128, S], F16, tag="e",
                                                  name=f"e{hh}"))
                        # ---- logits chunks + exp, heads interleaved so the
                        # two K=64 matmuls run on disjoint PE row groups ----
                        for ci in range(nch):
                            k0 = ci * CHUNK
                            kw = min(CHUNK, kext - k0)
                            ps_ls = [psL.tile([128, CHUNK], F32, tag="l",
                                              name=f"l{hh}") for hh in range(2)]
                            for sub in range((kw + 511) // 512):
                                s0 = sub * 512
                                sw = min(512, kw - s0)
                                last = (ci == nch - 1) and (s0 + sw == kw)
                                for hh in range(2):
                                    po = 64 * hh
                                    nc.tensor.matmul(
                                        ps_ls[hh][:, s0:s0 + sw],
                                        qhT[po:po + 64, g, qt * 128:(qt + 1) * 128],
                                        khT[po:po + 64, g, k0 + s0:k0 + s0 + sw],
                                        start=True, stop=not last)
                                if last:
                                    for hh in range(2):
                                        nc.tensor.matmul(
                                            ps_ls[hh][:, kw - 128:kw], ident_h[:],
                                            maskd_sb[:], start=False, stop=True)
                            for hh in range(2):
                                nc.scalar.activation(
                                    e_ts[hh][:, k0:k0 + kw], ps_ls[hh][:, :kw],
                                    AF.Exp, scale=0.125)
                        # ---- transpose + attn@vh (unnormalised exp) ----
                        for hh in range(2):
                            h = 2 * g + hh
                            e_t = e_ts[hh]
                            ps_av = psAV.tile([128, VW], F32, tag="av",
                                              name=f"av{hh}")
                            for g0 in range(0, nblk, 4):
                                gn = min(4, nblk - g0)
                                ps_t = psT.tile([128, 512], F16, tag="t")
                                for bi in range(gn):
                                    nc.tensor.transpose(
                                        ps_t[:, bi * 128:(bi + 1) * 128],
                                        e_t[:, (g0 + bi) * 128:(g0 + bi + 1) * 128],
                                        ident_h[:])
                                eT_sb = eTp.tile([128, 512], F16, tag="eT")
                                nc.vector.tensor_copy(eT_sb[:, :gn * 128],
                                                      ps_t[:, :gn * 128])
                                for bi in range(gn):
                                    kb = g0 + bi
                                    nc.tensor.matmul(
                                        ps_av[:],
                                        eT_sb[:, bi * 128:(bi + 1) * 128],
                                        vh[:, kb, h, :],
                                        start=(kb == 0), stop=(kb == nblk - 1))
                            # ---- softmax tail: row sum is ps_av[:, 64] ----
                            rc = stats.tile([128, 1], F32, tag="rc")
                            nc.vector.reciprocal(rc[:], ps_av[:, 64:65])
                            a_t = arow.tile([128, S], F16, tag="a")
                            nc.vector.tensor_scalar_mul(a_t[:, :kext],
                                                        e_t[:, :kext], rc[:])
                            nc.sync.dma_start(
                                attn[h, qt * 128:(qt + 1) * 128, 0:kext],
                                a_t[:, :kext])
                            nc.vector.tensor_scalar_mul(
                                c_t[:, h * 64:(h + 1) * 64], ps_av[:, 0:64], rc[:])
                    # ---- context transpose + output projection ----
                    for jj in range(2):
                        ps_ct = psT.tile([128, 512], F16, tag="t")
                        nc.tensor.transpose(
                            ps_ct[:, :128],
                            c_t[:, jj * 128:(jj + 1) * 128],
                            ident_h[:])
                        nc.scalar.copy(
                            ctT[:, jj, qt * 128:(qt + 1) * 128], ps_ct[:, :128])
                    for n in range(D // 512):
                        ps_o = psT.tile([128, 512], F32, tag="t")
                        for jj in range(2):
                            nc.tensor.matmul(
                                ps_o[:],
                                ctT[:, jj, qt * 128:(qt + 1) * 128],
                                wo_sb[:, jj, n * 512:(n + 1) * 512],
                                start=(jj == 0), stop=(jj == 1))
                        o_t = eTp.tile([128, 512], F32, tag="ot")
                        nc.scalar.copy(o_t[:], ps_o[:])
                        nc.sync.dma_start(
                            pout[qt * 128:(qt + 1) * 128, n * 512:(n + 1) * 512],
                            o_t[:])

    nc.compile()
    return nc


def _make_in_maps(v, k, q, mask, wq_k, wq_b, wk_k, wk_b, wv_k, wv_b, wo_k):
    maskd = np.ascontiguousarray(mask[0, 0, :128, :128]
                                 * np.float32(MASKVAL)).astype(np.float16)
    in_maps = []
    for c in range(NCORES):
        b = c // 4
        h0 = (c % 4) * HPC
        cs, ce = h0 * DEPTH, (h0 + HPC) * DEPTH
        in_maps.append({
            "xqT": np.ascontiguousarray(q[b].T).astype(np.float16),
            "xkT": np.ascontiguousarray(k[b].T).astype(np.float16),
            "xvT": np.ascontiguousarray(v[b].T).astype(np.float16),
            "wq": np.ascontiguousarray(wq_k[:, cs:ce]).astype(np.float16),
            "wk": np.ascontiguousarray(wk_k[:, cs:ce]).astype(np.float16),
            "wv": np.ascontiguousarray(wv_k[:, cs:ce]).astype(np.float16),
            "wqb": np.ascontiguousarray(wq_b[cs:ce].reshape(2, 128)),
            "wkb": np.ascontiguousarray(wk_b[cs:ce].reshape(2, 128)),
            "wvb": np.ascontiguousarray(wv_b[cs:ce].reshape(1, DH)).astype(np.float16),
            "wo": np.ascontiguousarray(wo_k[cs:ce].reshape(2, 128, D)).astype(np.float16),
            "maskd": maskd,
        })
    return in_maps


def kernel(v, k, q, mask, wq_k, wq_b, wk_k, wk_b, wv_k, wv_b, wo_k, wo_b,
           ar_w, ma_w, alpha, beta, _trace=False):
    v = np.asarray(v, np.float32)
    k = np.asarray(k, np.float32)
    q = np.asarray(q, np.float32)
    mask = np.asarray(mask, np.float32)
    wq_k = np.asarray(wq_k, np.float32)
    wk_k = np.asarray(wk_k, np.float32)
    wv_k = np.asarray(wv_k, np.float32)
    wo_k = np.asarray(wo_k, np.float32)
    wq_b = np.asarray(wq_b, np.float32)
    wk_b = np.asarray(wk_b, np.float32)
    wv_b = np.asarray(wv_b, np.float32)
    wo_b = np.asarray(wo_b, np.float32)

    if _trace:
        _install_ntff_hook()
    if "nc" not in _CACHE:
        _CACHE["nc"] = _build()
    nc = _CACHE["nc"]

    in_maps = _make_in_maps(v, k, q, mask, wq_k, wq_b, wk_k, wk_b,
                            wv_k, wv_b, wo_k)
    res = run_bass_kernel_spmd(nc, in_maps, list(range(NCORES)), trace=_trace)
    if _trace:
        print(f"HW exec time: {res.exec_time_ns} ns", flush=True)
        _CACHE["exec_time_ns"] = res.exec_time_ns
        _CACHE["results"] = res

    attn_full = np.concatenate(
        [res.results[c]["attn"][None].astype(np.float32) for c in range(NCORES)]
    ).reshape(B, H, S, S)
    out = np.stack([
        sum(res.results[c]["pout"] for c in range(4 * b, 4 * b + 4)) + wo_b
        for b in range(B)
    ]).astype(np.float32)
    return out, attn_full


# revision 22
# speedup vs baseline: 4.2993x; 1.0602x over previous
"""ARMA multi-head attention TRN2 kernel (nn_ARMAMultiHeadAttention).

Problem: B=2, S=2048, D=1024, H=16 heads of depth 64, causal mask.
  qh/kh/vh = split_heads(x @ W + b);  logits = qh@kh^T/8 + mask*(-1e9)
  (+ alpha*ar + beta*ma per-QUERY-row bias, which is softmax-shift-invariant
   and therefore has no effect on the outputs -> skipped on device)
  attn = softmax(logits);  out = (attn @ vh merged) @ wo + wo_b
Returns (out, attn) like the reference.

Sharding: 8 cores; core c handles batch b=c//4 and heads 4*(c%4)..4*(c%4)+3
(data + head parallel). wq/wk/wv column-sharded, wo row-sharded; per-batch
partial outputs summed on the host (the unshard step of row-parallel wo).

Numerics: fp16 on the PE-heavy paths (1 cyc/row vs 4 for fp32; values all
within fp16 range; ~2.4e-4 rounding), fp32 PSUM accumulation everywhere,
fp32 softmax reciprocal. Mask bias is -6e4 (fits fp16; exp underflows to 0
exactly, same as the reference's -1e9). attn is stored fp16 and widened on
the host.

Per-core flow:
  - projections: qh^T,kh^T [256,2048] fp16; vh [2048, 4 heads x 66] fp16
    with a ones column per head (fuses the softmax row-sum into attn@vh)
  - per (q-tile, head pair): causal logits chunks in PSUM (two heads
    interleaved on disjoint PE row groups), -6e4 on the diagonal block via
    an identity matmul, Exp -> e_t fp16, PE-transpose of e blocks -> attn@vh
    (N=66: col 64 = row sum), reciprocal, normalize+store attn, scale
    context, context^T, wo row-shard matmul streamed per q-tile.
"""

import os
import sys
import types
import numpy as np
import ml_dtypes

import concourse.bass as bass
import concourse.mybir as mybir
import concourse.tile as tile
from concourse import bacc
from concourse.bass_utils import run_bass_kernel_spmd
from concourse.masks import make_identity

F32 = mybir.dt.float32
F16 = mybir.dt.float16
AF = mybir.ActivationFunctionType

B, S, D, H = 2, 2048, 1024, 16
DEPTH = D // H          # 64
NCORES = 8
HPC = H // 4            # heads per core = 4
DH = HPC * DEPTH        # 256 per-core projection width
NQT = S // 128          # 16 q tiles
KC = D // 128           # 8 contraction chunks for projections
CHUNK = 1024            # logits psum chunk (2 banks)
VW = 66                 # vh cols per head: 64 data + ones + pad
MASKVAL = -60000.0      # "-inf" that fits fp16; exp(0.125*x) underflows to 0

_CACHE = {}


def _install_ntff_hook():
    """The agent image's antenv lacks axon_hooks; register the NTFF profile
    hook manually so run_bass_kernel_spmd(trace=True) works under axon."""
    import antenv
    if "antenv.axon_hooks" in sys.modules:
        return
    mod = types.ModuleType("antenv.axon_hooks")
    _hook = [None]
    mod.set_axon_ntff_profile_hook = lambda h: _hook.__setitem__(0, h)
    mod.get_axon_ntff_profile_hook = lambda: _hook[0]
    sys.modules["antenv.axon_hooks"] = mod
    antenv.axon_hooks = mod
    from trn_agent_boot.trn_boot import _ntff_profile_via_ctypes
    mod.set_axon_ntff_profile_hook(
        _ntff_profile_via_ctypes("/opt/axon/libaxon_pjrt.so"))


def _build():
    nc = bacc.Bacc("TRN2", target_bir_lowering=False, debug=False,
                   enable_asserts=False)

    xqT = nc.dram_tensor("xqT", [D, S], F16, kind="ExternalInput").ap()
    xkT = nc.dram_tensor("xkT", [D, S], F16, kind="ExternalInput").ap()
    xvT = nc.dram_tensor("xvT", [D, S], F16, kind="ExternalInput").ap()
    wq = nc.dram_tensor("wq", [D, DH], F16, kind="ExternalInput").ap()
    wk = nc.dram_tensor("wk", [D, DH], F16, kind="ExternalInput").ap()
    wv = nc.dram_tensor("wv", [D, DH], F16, kind="ExternalInput").ap()
    wqb = nc.dram_tensor("wqb", [2, 128], F32, kind="ExternalInput").ap()
    wkb = nc.dram_tensor("wkb", [2, 128], F32, kind="ExternalInput").ap()
    wvb = nc.dram_tensor("wvb", [1, DH], F16, kind="ExternalInput").ap()
    wo = nc.dram_tensor("wo", [2, 128, D], F16, kind="ExternalInput").ap()
    maskd = nc.dram_tensor("maskd", [128, 128], F16, kind="ExternalInput").ap()

    attn = nc.dram_tensor("attn", [HPC, S, S], F16, kind="ExternalOutput").ap()
    pout = nc.dram_tensor("pout", [S, D], F32, kind="ExternalOutput").ap()

    with tile.TileContext(nc) as tc:
        with (
            tc.tile_pool(name="persist", bufs=1) as persist,
            tc.tile_pool(name="small", bufs=1) as small,
        ):
            qhT = persist.tile([128, 2, S], F16, tag="qhT")
            khT = persist.tile([128, 2, S], F16, tag="khT")
            vh = persist.tile([128, NQT, HPC, VW], F16, tag="vh")
            ctT = persist.tile([128, 2, S], F16, tag="ctT")
            wo_sb = persist.tile([128, 2, D], F16, tag="wo")

            ident = small.tile([128, 128], F32)
            make_identity(nc, ident[:])
            ident_h = small.tile([128, 128], F16)
            make_identity(nc, ident_h[:])
            maskd_sb = small.tile([128, 128], F16)
            nc.sync.dma_start(maskd_sb[:], maskd)
            wqb_sb = small.tile([128, 2], F32)
            nc.sync.dma_start(wqb_sb[:], wqb.rearrange("j p -> p j"))
            wkb_sb = small.tile([128, 2], F32)
            nc.sync.dma_start(wkb_sb[:], wkb.rearrange("j p -> p j"))
            wvb_sb = small.tile([1, DH], F16)
            nc.sync.dma_start(wvb_sb[:], wvb)
            ones1 = small.tile([1, 128], F16)
            nc.vector.memset(ones1[:], 1.0)
            nc.sync.dma_start(wo_sb[:], wo.rearrange("j p n -> p j n"))
            # ones column (and pad) for the fused row-sum
            nc.vector.memset(vh[:, :, :, 64:65], 1.0)
            nc.vector.memset(vh[:, :, :, 65:66], 0.0)

            # ---------------- projections ----------------
            with (
                tc.tile_pool(name="xin", bufs=2) as xin,
                tc.tile_pool(name="wproj", bufs=1) as wproj,
                tc.tile_pool(name="ppsum", bufs=8, space="PSUM") as ppsum,
            ):
                wq_sb = wproj.tile([128, KC, DH], F16, tag="wq")
                wk_sb = wproj.tile([128, KC, DH], F16, tag="wk")
                wv_sb = wproj.tile([128, KC, DH], F16, tag="wv")
                nc.sync.dma_start(wq_sb[:], wq.rearrange("(c p) m -> p c m", p=128))
                nc.sync.dma_start(wk_sb[:], wk.rearrange("(c p) m -> p c m", p=128))
                nc.sync.dma_start(wv_sb[:], wv.rearrange("(c p) m -> p c m", p=128))

                # prefetch q and k inputs together (chunked) so the PE never
                # waits on DMA past the first chunk
                x_q = xin.tile([128, KC, S], F16, tag="x", name="x_q")
                x_k = xin.tile([128, KC, S], F16, tag="x", name="x_k")
                for x_sb_, src in ((x_q, xqT), (x_k, xkT)):
                    src_r = src.rearrange("(c p) s -> p c s", p=128)
                    for c in range(KC):
                        nc.sync.dma_start(x_sb_[:, c, :], src_r[:, c, :])

                for x_sb, wsb, bsb, dstT in (
                    (x_q, wq_sb, wqb_sb, qhT),
                    (x_k, wk_sb, wkb_sb, khT),
                ):
                    # all 8 output groups accumulate in parallel (8 PSUM
                    # banks) so PE streams chunk-by-chunk as DMA lands
                    pss = [ppsum.tile([128, 512], F32, tag="p", name=f"pp{i}")
                           for i in range(8)]
                    for c in range(KC):
                        for j in range(2):
                            for n in range(S // 512):
                                nc.tensor.matmul(
                                    pss[j * 4 + n][:],
                                    wsb[:, c, j * 128:(j + 1) * 128],
                                    x_sb[:, c, n * 512:(n + 1) * 512],
                                    start=(c == 0), stop=(c == KC - 1))
                    for j in range(2):
                        for n in range(S // 512):
                            nc.scalar.activation(
                                dstT[:, j, n * 512:(n + 1) * 512],
                                pss[j * 4 + n][:],
                                AF.Identity, bias=bsb[:, j:j + 1])

                # v projection: natural layout [s, per-head 66-col groups]
                x_sb = xin.tile([128, KC, S], F16, tag="x", name="x_v")
                xv_r = xvT.rearrange("(c p) s -> p c s", p=128)
                for c in range(KC):
                    nc.sync.dma_start(x_sb[:, c, :], xv_r[:, c, :])
                for st in range(NQT):
                    ps = ppsum.tile([128, 512], F32, tag="p")
                    for c in range(KC):
                        nc.tensor.matmul(
                            ps[:, :DH],
                            x_sb[:, c, st * 128:(st + 1) * 128],
                            wv_sb[:, c, :],
                            start=(c == 0), stop=False)
                    nc.tensor.matmul(ps[:, :DH], ones1[:], wvb_sb[:],
                                     start=False, stop=True)
                    nc.scalar.copy(
                        vh[:, st, :, 0:64],
                        ps[:, :DH].rearrange("p (h d) -> p h d", d=64))

            # ---------------- attention ----------------
            with (
                tc.tile_pool(name="erow", bufs=3) as erow,
                tc.tile_pool(name="arow", bufs=3) as arow,
                tc.tile_pool(name="eT", bufs=4) as eTp,
                tc.tile_pool(name="stats", bufs=4) as stats,
                tc.tile_pool(name="ctile", bufs=2) as ctile,
                tc.tile_pool(name="psL", bufs=2, space="PSUM") as psL,
                tc.tile_pool(name="psT", bufs=2, space="PSUM") as psT,
                tc.tile_pool(name="psAV", bufs=2, space="PSUM") as psAV,
            ):
                copy_flip = 0
                for qt in range(NQT):
                    kext = (qt + 1) * 128
                    nblk = qt + 1
                    nch = (kext + CHUNK - 1) // CHUNK
                    c_t = ctile.tile([128, DH], F16, tag="c")
                    for g in range(2):  # head pairs (2g, 2g+1), j = g
                        e_ts = []
                        ps_ls = []
                        for hh in range(2):
                            e_ts.append(erow.tile([128, S], F16, tag="e",
                                                  name=f"e{hh}"))
                        # ---- logits chunks + exp, heads interleaved so the
                        # two K=64 matmuls run on disjoint PE row groups ----
                        for ci in range(nch):
                            k0 = ci * CHUNK
                            kw = min(CHUNK, kext - k0)
                            ps_ls = [psL.tile([128, CHUNK], F32, tag="l",
                                              name=f"l{hh}") for hh in range(2)]
                            for sub in range((kw + 511) // 512):
                                s0 = sub * 512
                                sw = min(512, kw - s0)
                                last = (ci == nch - 1) and (s0 + sw == kw)
                                for hh in range(2):
                                    po = 64 * hh
                                    nc.tensor.matmul(
                                        ps_ls[hh][:, s0:s0 + sw],
                                        qhT[po:po + 64, g, qt * 128:(qt + 1) * 128],
                                        khT[po:po + 64, g, k0 + s0:k0 + s0 + sw],
                                        start=True, stop=not last)
                                if last:
                                    for hh in range(2):
                                        nc.tensor.matmul(
                                            ps_ls[hh][:, kw - 128:kw], ident_h[:],
                                            maskd_sb[:], start=False, stop=True)
                            for hh in range(2):
                                nc.scalar.activation(
                                    e_ts[hh][:, k0:k0 + kw], ps_ls[hh][:, :kw],
                                    AF.Exp, scale=0.125)
                        # ---- transpose + attn@vh (unnormalised exp) ----
                        for hh in range(2):
                            h = 2 * g + hh
                            e_t = e_ts[hh]
                            ps_av = psAV.tile([128, VW], F32, tag="av",
                                              name=f"av{hh}")
                            for g0 in range(0, nblk, 4):
                                gn = min(4, nblk - g0)
                                ps_t = psT.tile([128, 512], F16, tag="t")
                                for bi in range(gn):
                                    nc.tensor.transpose(
                                        ps_t[:, bi * 128:(bi + 1) * 128],
                                        e_t[:, (g0 + bi) * 128:(g0 + bi + 1) * 128],
                                        ident_h[:])
                                eT_sb = eTp.tile([128, 512], F16, tag="eT")
                                nc.vector.tensor_copy(eT_sb[:, :gn * 128],
                                                      ps_t[:, :gn * 128])
                                for bi in range(gn):
                                    kb = g0 + bi
                                    nc.tensor.matmul(
                                        ps_av[:],
                                        eT_sb[:, bi * 128:(bi + 1) * 128],
                                        vh[:, kb, h, :],
                                        start=(kb == 0), stop=(kb == nblk - 1))
                            # ---- softmax tail: row sum is ps_av[:, 64] ----
                            rc = stats.tile([128, 1], F32, tag="rc")
                            nc.vector.reciprocal(rc[:], ps_av[:, 64:65])
                            a_t = arow.tile([128, S], F16, tag="a")
                            nc.vector.tensor_scalar_mul(a_t[:, :kext],
                                                        e_t[:, :kext], rc[:])
                            nc.sync.dma_start(
                                attn[h, qt * 128:(qt + 1) * 128, 0:kext],
                                a_t[:, :kext])
                            nc.vector.tensor_scalar_mul(
                                c_t[:, h * 64:(h + 1) * 64], ps_av[:, 0:64], rc[:])
                    # ---- context transpose + output projection ----
                    for jj in range(2):
                        ps_ct = psT.tile([128, 512], F16, tag="t")
                        nc.tensor.transpose(
                            ps_ct[:, :128],
                            c_t[:, jj * 128:(jj + 1) * 128],
                            ident_h[:])
                        nc.scalar.copy(
                            ctT[:, jj, qt * 128:(qt + 1) * 128], ps_ct[:, :128])
                    for n in range(D // 512):
                        ps_o = psT.tile([128, 512], F32, tag="t")
                        for jj in range(2):
                            nc.tensor.matmul(
                                ps_o[:],
                                ctT[:, jj, qt * 128:(qt + 1) * 128],
                                wo_sb[:, jj, n * 512:(n + 1) * 512],
                                start=(jj == 0), stop=(jj == 1))
                        o_t = eTp.tile([128, 512], F32, tag="ot")
                        nc.scalar.copy(o_t[:], ps_o[:])
                        nc.sync.dma_start(
                            pout[qt * 128:(qt + 1) * 128, n * 512:(n + 1) * 512],
                            o_t[:])

    nc.compile()
    return nc


def _make_in_maps(v, k, q, mask, wq_k, wq_b, wk_k, wk_b, wv_k, wv_b, wo_k):
    maskd = np.ascontiguousarray(mask[0, 0, :128, :128]
                                 * np.float32(MASKVAL)).astype(np.float16)
    in_maps = []
    for c in range(NCORES):
        b = c // 4
        h0 = (c % 4) * HPC
        cs, ce = h0 * DEPTH, (h0 + HPC) * DEPTH
        in_maps.append({
            "xqT": np.ascontiguousarray(q[b].T).astype(np.float16),
            "xkT": np.ascontiguousarray(k[b].T).astype(np.float16),
            "xvT": np.ascontiguousarray(v[b].T).astype(np.float16),
            "wq": np.ascontiguousarray(wq_k[:, cs:ce]).astype(np.float16),
            "wk": np.ascontiguousarray(wk_k[:, cs:ce]).astype(np.float16),
            "wv": np.ascontiguousarray(wv_k[:, cs:ce]).astype(np.float16),
            "wqb": np.ascontiguousarray(wq_b[cs:ce].reshape(2, 128)),
            "wkb": np.ascontiguousarray(wk_b[cs:ce].reshape(2, 128)),
            "wvb": np.ascontiguousarray(wv_b[cs:ce].reshape(1, DH)).astype(np.float16),
            "wo": np.ascontiguousarray(wo_k[cs:ce].reshape(2, 128, D)).astype(np.float16),
            "maskd": maskd,
        })
    return in_maps


def kernel(v, k, q, mask, wq_k, wq_b, wk_k, wk_b, wv_k, wv_b, wo_k, wo_b,
           ar_w, ma_w, alpha, beta, _trace=False):
    v = np.asarray(v, np.float32)
    k = np.asarray(k, np.float32)
    q = np.asarray(q, np.float32)
    mask = np.asarray(mask, np.float32)
    wq_k = np.asarray(wq_k, np.float32)
    wk_k = np.asarray(wk_k, np.float32)
    wv_k = np.asarray(wv_k, np.float32)
    wo_k = np.asarray(wo_k, np.float32)
    wq_b = np.asarray(wq_b, np.float32)
    wk_b = np.asarray(wk_b, np.float32)
    wv_b = np.asarray(wv_b, np.float32)
    wo_b = np.asarray(wo_b, np.float32)

    if _trace:
        _install_ntff_hook()
    if "nc" not in _CACHE:
        _CACHE["nc"] = _build()
    nc = _CACHE["nc"]

    in_maps = _make_in_maps(v, k, q, mask, wq_k, wq_b, wk_k, wk_b,
                            wv_k, wv_b, wo_k)
    res = run_bass_kernel_spmd(nc, in_maps, list(range(NCORES)), trace=_trace)
    if _trace:
        print(f"HW exec time: {res.exec_time_ns} ns", flush=True)
        _CACHE["exec_time_ns"] = res.exec_time_ns
        _CACHE["results"] = res

    attn_full = np.concatenate(
        [res.results[c]["attn"][None].astype(np.float32) for c in range(NCORES)]
    ).reshape(B, H, S, S)
    out = np.stack([
        sum(res.results[c]["pout"] for c in range(4 * b, 4 * b + 4)) + wo_b
        for b in range(B)
    ]).astype(np.float32)
    return out, attn_full
